# revision 1
# baseline (speedup 1.0000x reference)
"""Trainium2 Bass kernel for DepST_RNN (dependency-tree GNN message passing).

Contract: kernel(**inputs) takes FULL inputs, returns FULL output
[B, N, NODE+DEP] float32.  One NeuronCore per sentence (B=8 data-parallel).

Device algorithm per core (one sentence):
  * ctx pass: all L*E per-edge ctx messages Wc[rel] @ ctx[tail], batched
    relation-major so each Wc[r] loads into the PE array once.  Result is
    transposed into token-rows (msgcT) for later per-layer SWDGE gathers.
  * recursion over L layers: gather child vectors (dma_gather from a bf16
    token history addressed by host-computed provenance), 40 relation-slot
    matmuls Wd[r] @ child_tail, merge with gathered ctx messages, PE
    transpose to edge-rows, scale by host-computed mask/count factors,
    dma_scatter_add (f32 CCE accumulate) into per-layer sums, cast to the
    bf16 history.
  * final: provenance gather assembles child^T, DMA'd out; host transposes
    and concatenates with context.

All data-dependent structure (relation grouping, provenance, scatter
targets, mean scales) is computed on host from the integer index tensors
and shipped as data.  The instruction structure is made identical across
the 8 cores by max-enveloping relation-slot sizes over cores (SPMD: one
program, per-core data).
"""

import sys

sys.path.insert(0, "/opt/trn_rl_repo")

from contextlib import ExitStack

import numpy as np
import ml_dtypes

import concourse.bass as bass
import concourse.bacc as bacc
import concourse.mybir as mybir
from concourse import tile
from concourse.bass_utils import run_bass_kernel_spmd

B, L, E, N = 8, 8, 128, 1024
NODE, DEP, R = 256, 128, 40

BF16 = mybir.dt.bfloat16
F32 = mybir.dt.float32
I16 = mybir.dt.int16

NPBF16 = ml_dtypes.bfloat16

STAGE = 99  # debug bisect: 0=io,1=+ctx,2=+transposes,3+=n layers
ZRANK_CH = 64 * 128  # childhist zero-rank token base (layers use ranks 0..63)


def _wrap_idx(idx):
    """[n] int -> [128, n//16] int16, 16-partition wrap replicated 8x."""
    idx = np.asarray(idx, np.int64)
    n = idx.shape[0]
    assert n % 16 == 0, n
    w = idx.reshape(n // 16, 16).T.astype(np.int16)  # [16, n/16]; w[p,s] = idx[16s+p]
    return np.tile(w, (8, 1))


def prep(context, dep_W, heads, tails, rels, mask):
    """Host-side structure + per-core input tensors."""
    ctx_np = np.asarray(context, np.float32)
    W_np = np.asarray(dep_W, np.float32)
    heads = np.asarray(heads)
    tails = np.asarray(tails)
    rels = np.asarray(rels)
    mask_np = np.asarray(mask, np.float32)

    # --- per-(core, layer) relation-sorted edge order and counts ---
    order = np.zeros((B, L, E), np.int64)
    cnt = np.zeros((B, L, R), np.int64)
    for b in range(B):
        for l in range(L):
            order[b, l] = np.argsort(rels[b, l], kind="stable")
            cnt[b, l] = np.bincount(rels[b, l], minlength=R)

    # --- cross-core envelopes: shared instruction structure ---
    cmax = cnt.max(axis=0)  # [L, R] layer slot sizes
    E_real = cmax.sum(axis=1)  # [L]
    assert (E_real <= 512).all(), f"layer envelope > 512: {E_real}"
    NBLK = [max(1, int(np.ceil(e / 128))) for e in E_real]
    WL = [nb * 128 for nb in NBLK]
    loff = np.zeros((L, R), np.int64)
    for l in range(L):
        loff[l, 1:] = np.cumsum(cmax[l])[:-1]

    gcnt = cnt.sum(axis=1)  # [B, R] per-core global relation counts
    genv = gcnt.max(axis=0)  # [R]
    goff = np.zeros(R, np.int64)
    goff[1:] = np.cumsum(genv)[:-1]
    Gpad = int(genv.sum())
    NGBLK = int(np.ceil(Gpad / 128))
    GW = NGBLK * 128

    # scatter waves: nsc[l] = ceil(max-head-multiplicity / 2) across cores
    maxmult = np.zeros(L, np.int64)
    for b in range(B):
        for l in range(L):
            mm = np.bincount(heads[b, l], minlength=N).max()
            maxmult[l] = max(maxmult[l], mm)
    nsc = [max(1, int(np.ceil(m / 2))) for m in maxmult]

    # idx tensor layout: [gidx(l=0..L-1) | cidx(l) | sidx(l, wave w) | fidx]
    sec_w = [w // 16 for w in WL]
    g_sec = np.zeros(L, np.int64)
    for l in range(1, L):
        g_sec[l] = g_sec[l - 1] + sec_w[l - 1]
    total_w = int(sum(sec_w))
    c_sec = g_sec + total_w
    s_sec = []  # s_sec[l][w]
    pos = 2 * total_w
    for l in range(L):
        s_sec.append([pos + w * sec_w[l] for w in range(nsc[l])])
        pos += nsc[l] * sec_w[l]
    f_sec = pos
    IW = pos + 1024 // 16

    st = dict(
        cmax=cmax, E_real=E_real, NBLK=NBLK, WL=WL, loff=loff,
        genv=genv, goff=goff, Gpad=Gpad, NGBLK=NGBLK, GW=GW,
        g_sec=g_sec, c_sec=c_sec, s_sec=s_sec, f_sec=f_sec, IW=IW, nsc=nsc,
    )

    # --- shared weight layouts ---
    wc_np = np.zeros((128, 2 * R * 128), np.float32)
    wd_np = np.zeros((128, R * 128), np.float32)
    for r in range(R):
        for c in range(2):
            wc_np[:, (c * R + r) * 128:(c * R + r + 1) * 128] = (
                W_np[r, :, c * 128:(c + 1) * 128].T
            )
        wd_np[:, r * 128:(r + 1) * 128] = W_np[r, :, NODE:].T
    wc_np = wc_np.astype(NPBF16)
    wd_np = wd_np.astype(NPBF16)
    ident_np = np.eye(128, dtype=np.float32)

    # --- per-core tables ---
    in_maps = []
    for b in range(B):
        prov = np.full(N, -1, np.int64)
        provs = []
        cvals = []
        for l in range(L):
            provs.append(prov.copy())
            c = np.zeros(N, np.float32)
            np.add.at(c, heads[b, l], mask_np[b, l])
            cvals.append(c)
            prov = np.where(c > 0, l, prov)
        provs.append(prov.copy())

        ctxg = np.zeros((2 * 128, GW), np.float32)
        s_np = np.zeros((128, 4 * L), np.float32)
        gidx, cidx, sidx = [], [], []
        gfill = goff.copy()
        for l in range(L):
            W_l = WL[l]
            ar = np.arange(W_l)
            gi = ZRANK_CH + (ar % 128)
            ci = NGBLK * 128 + (ar % 128)
            # per-wave scatter tables; default -> trash cells (group 8)
            si = [16 * 128 + (ar % 128) for _ in range(nsc[l])]
            occ_cnt = {}
            for r in range(R):
                es = [e for e in order[b, l] if rels[b, l, e] == r]
                for k, e in enumerate(es):
                    j = int(loff[l, r]) + k
                    t = int(tails[b, l, e])
                    h = int(heads[b, l, e])
                    m = float(mask_np[b, l, e])
                    p = int(provs[l][t])
                    if p >= 0:
                        gi[j] = p * 1024 + t
                    g = int(gfill[r])
                    gfill[r] += 1
                    ci[j] = g
                    ctxg[:, g] = ctx_np[b, t, :]
                    o = occ_cnt.get(h, 0)
                    occ_cnt[h] = o + 1
                    si[o // 2][j] = (h // 128) * 256 + (o % 2) * 128 + (h % 128)
                    s_np[j % 128, 4 * l + j // 128] = m / max(float(cvals[l][h]), 1.0)
            # each wave's real dst cells must be unique (race-freedom on HW)
            for w in range(nsc[l]):
                real = si[w][si[w] < 16 * 128]
                assert len(np.unique(real)) == len(real)
            gidx.append(gi)
            cidx.append(ci)
            sidx.extend(si)

        fi = np.where(
            provs[L] >= 0,
            provs[L] * 1024 + np.arange(N),
            ZRANK_CH + (np.arange(N) % 128),
        )
        idx_np = np.concatenate(
            [_wrap_idx(x) for x in (gidx + cidx + sidx + [fi])], axis=1
        )
        assert idx_np.shape == (128, IW)

        in_maps.append(
            dict(
                ctxg=ctxg.astype(NPBF16),
                wc=wc_np,
                wd=wd_np,
                s=s_np,
                idx=idx_np,
                ident=ident_np,
            )
        )
    return st, in_maps


def build(nc, st):
    GW, NGBLK = st["GW"], st["NGBLK"]
    cmax, E_real, NBLK, WL, loff = (
        st["cmax"], st["E_real"], st["NBLK"], st["WL"], st["loff"],
    )
    genv, goff = st["genv"], st["goff"]

    d_ctxg = nc.declare_dram_parameter("ctxg", [256, GW], BF16, isOutput=False)
    d_wc = nc.declare_dram_parameter("wc", [128, 2 * R * 128], BF16, isOutput=False)
    d_wd = nc.declare_dram_parameter("wd", [128, R * 128], BF16, isOutput=False)
    d_s = nc.declare_dram_parameter("s", [128, 4 * L], F32, isOutput=False)
    d_idx = nc.declare_dram_parameter("idx", [128, st["IW"]], I16, isOutput=False)
    d_ident = nc.declare_dram_parameter("ident", [128, 128], F32, isOutput=False)
    d_out = nc.declare_dram_parameter("childT", [128, 1024], BF16, isOutput=True)

    with ExitStack() as ctx:
        tc = ctx.enter_context(tile.TileContext(nc))

        pers = ctx.enter_context(tc.tile_pool(name="pers", bufs=1))

        def sb(name, shape, dt):
            return pers.tile(shape, dt, tag=name, name=name)

        ctxg0 = sb("ctxg0", [128, GW], BF16)
        ctxg1 = sb("ctxg1", [128, GW], BF16)
        wc = sb("wc_sb", [128, 2 * R * 128], BF16)
        wd = sb("wd_sb", [128, R * 128], BF16)
        s_sb = sb("s_sb", [128, 4 * L], F32)
        idx_sb = sb("idx_sb", [128, st["IW"]], I16)
        ident = sb("ident_sb", [128, 128], F32)
        msgc = sb("msgc", [128, GW], F32)
        msgcT = sb("msgcT", [128, (NGBLK + 1) * 128], BF16)
        chist = sb("chist", [128, 65 * 128], BF16)
        sums = sb("sums", [128, L * 1152], F32)
        sums_p = sb("sums_p", [128, L * 1152], F32)

        pool = ctx.enter_context(tc.tile_pool(name="work", bufs=3))
        pp_msg = ctx.enter_context(tc.tile_pool(name="ps_msg", bufs=2, space="PSUM"))
        pp_t = ctx.enter_context(tc.tile_pool(name="ps_t", bufs=2, space="PSUM"))
        pp_c = ctx.enter_context(tc.tile_pool(name="ps_c", bufs=1, space="PSUM"))

        # ---- input DMAs ----
        nc.sync.dma_start(ctxg0[:, :], d_ctxg[0:128, :])
        nc.sync.dma_start(ctxg1[:, :], d_ctxg[128:256, :])
        nc.sync.dma_start(wc[:, :], d_wc[:, :])
        nc.sync.dma_start(wd[:, :], d_wd[:, :])
        nc.sync.dma_start(s_sb[:, :], d_s[:, :])
        nc.sync.dma_start(idx_sb[:, :], d_idx[:, :])
        nc.sync.dma_start(ident[:, :], d_ident[:, :])

        # ---- zero init (sums accumulators, history, zero-rank tokens) ----
        nc.vector.memset(sums[:, :], 0.0)
        nc.vector.memset(sums_p[:, :], 0.0)
        nc.vector.memset(chist[:, :], 0.0)
        nc.vector.memset(msgcT[:, NGBLK * 128:], 0.0)

        # ---- ctx pass: relation-major Wc matmuls over all L*E edges ----
        if STAGE < 1:
            fin0 = pool.tile([128, 1024], BF16, tag="fin", name="finT0")
            for c in range(2):
                nc.gpsimd.dma_gather(
                    fin0[:, c * 512:(c + 1) * 512].rearrange("p (o w) -> p o w", o=1),
                    chist[:, :],
                    idx_sb[:, st["f_sec"] + c * 32:st["f_sec"] + (c + 1) * 32],
                    512, 512, 128,
                    transpose=True,
                    sbuf_tokens_per_rank=128,
                    sbuf_free_dim_per_rank=256,
                )
            nc.sync.dma_start(d_out[:, :], fin0[:, :])
            return nc
        nct = int(np.ceil(GW / 512))
        ctxps = [
            pp_c.tile([128, min(512, GW - 512 * i)], F32, tag=f"ctxps{i}", name=f"ctxps{i}")
            for i in range(nct)
        ]
        for r in range(R):
            a, width = int(goff[r]), int(genv[r])
            while width > 0:
                ti, off = a // 512, a % 512
                pw = min(width, 512 - off, GW - 512 * ti - off)
                src = (ctxg0, ctxg1)
                for c in (0, 1):
                    nc.tensor.matmul(
                        ctxps[ti][:, off:off + pw],
                        wc[:, (c * R + r) * 128:(c * R + r + 1) * 128],
                        src[c][:, a:a + pw],
                        start=(c == 0),
                        stop=(c == 1),
                    )
                a += pw
                width -= pw
        if st["Gpad"] < GW:
            # pad columns: write zeros (ctxg pad cols are zero)
            ti, off = st["Gpad"] // 512, st["Gpad"] % 512
            nc.tensor.matmul(
                ctxps[ti][:, off:],
                wc[:, 0:128],
                ctxg0[:, st["Gpad"]:GW],
                start=True,
                stop=True,
            )
        for i in range(nct):
            tw = min(512, GW - 512 * i)
            nc.vector.tensor_copy(msgc[:, 512 * i:512 * i + tw], ctxps[i][:, :tw])
        if STAGE < 2:
            fin0 = pool.tile([128, 1024], BF16, tag="fin", name="finT0")
            nc.vector.tensor_copy(fin0[:, :], msgc[:, :1024])
            nc.sync.dma_start(d_out[:, :], fin0[:, :])
            return nc
        # transpose msgc columns into token rows (bf16)
        for t in range(NGBLK):
            tp = pp_t.tile([128, 128], F32, tag="tp", name="tp")
            nc.tensor.transpose(tp[:, :], msgc[:, 128 * t:128 * (t + 1)], ident[:, :])
            nc.vector.tensor_copy(msgcT[:, 128 * t:128 * (t + 1)], tp[:, :])

        # ---- recursion over layers ----
        nlayers = max(0, min(L, STAGE - 2))
        if nlayers == 0:
            fin0 = pool.tile([128, 1024], BF16, tag="fin", name="finT0")
            nc.vector.tensor_copy(fin0[:, :], msgcT[:, :1024])
            nc.sync.dma_start(d_out[:, :], fin0[:, :])
            return nc
        for l in range(nlayers):
            W_l, nb, er = WL[l], NBLK[l], int(E_real[l])
            G = pool.tile([128, W_l], BF16, tag="G", name="G")
            cT = pool.tile([128, W_l], BF16, tag="cT", name="cT")
            nc.gpsimd.dma_gather(
                G[:, :].rearrange("p (o w) -> p o w", o=1),
                chist[:, :],
                idx_sb[:, st["g_sec"][l]:st["g_sec"][l] + W_l // 16],
                W_l, W_l, 128,
                transpose=True,
                sbuf_tokens_per_rank=128,
                sbuf_free_dim_per_rank=256,
            )
            nc.gpsimd.dma_gather(
                cT[:, :].rearrange("p (o w) -> p o w", o=1),
                msgcT[:, :],
                idx_sb[:, st["c_sec"][l]:st["c_sec"][l] + W_l // 16],
                W_l, W_l, 128,
                transpose=True,
                sbuf_tokens_per_rank=128,
                sbuf_free_dim_per_rank=256,
            )
            mps = pp_msg.tile([128, W_l], F32, tag="mps", name="mps")
            for r in range(R):
                cm = int(cmax[l, r])
                if cm == 0:
                    continue
                off = int(loff[l, r])
                nc.tensor.matmul(
                    mps[:, off:off + cm],
                    wd[:, r * 128:(r + 1) * 128],
                    G[:, off:off + cm],
                    start=True,
                    stop=True,
                )
            if er < W_l:
                nc.tensor.matmul(
                    mps[:, er:W_l],
                    wd[:, 0:128],
                    G[:, er:W_l],
                    start=True,
                    stop=True,
                )
            tmp = pool.tile([128, W_l], F32, tag="tmp", name="tmp")
            nc.vector.tensor_add(tmp[:, :], mps[:, :], cT[:, :])
            msgS = pool.tile([128, W_l], F32, tag="msgS", name="msgS")
            for t in range(nb):
                tp = pp_t.tile([128, 128], F32, tag="tp", name="tp")
                nc.tensor.transpose(
                    tp[:, :], tmp[:, 128 * t:128 * (t + 1)], ident[:, :]
                )
                nc.vector.tensor_scalar(
                    msgS[:, 128 * t:128 * (t + 1)],
                    tp[:, :],
                    s_sb[:, 4 * l + t:4 * l + t + 1],
                    None,
                    mybir.AluOpType.mult,
                )
            for w in range(st["nsc"][l]):
                sec = st["s_sec"][l][w]
                nc.gpsimd.dma_scatter_add(
                    sums[:, l * 1152:(l + 1) * 1152],
                    msgS[:, :].rearrange("p (b d) -> p b d", d=128),
                    idx_sb[:, sec:sec + W_l // 16],
                    W_l, W_l, 128,
                    sbuf_tokens_per_rank=128,
                    parity_reg=0,
                    out_ap_other=sums_p[:, l * 1152:(l + 1) * 1152],
                )
            nc.vector.tensor_add(
                chist[:, l * 1024:(l + 1) * 1024],
                sums[:, l * 1152:l * 1152 + 1024],
                sums_p[:, l * 1152:l * 1152 + 1024],
            )

        # ---- final provenance gather + output ----
        # (dma_gather num_idxs > 512 fails on HW; chunk by 512)
        finT = pool.tile([128, 1024], BF16, tag="fin", name="finT")
        for c in range(2):
            nc.gpsimd.dma_gather(
                finT[:, c * 512:(c + 1) * 512].rearrange("p (o w) -> p o w", o=1),
                chist[:, :],
                idx_sb[:, st["f_sec"] + c * 32:st["f_sec"] + (c + 1) * 32],
                512, 512, 128,
                transpose=True,
                sbuf_tokens_per_rank=128,
                sbuf_free_dim_per_rank=256,
            )
        nc.sync.dma_start(d_out[:, :], finT[:, :])
    return nc


def run(inputs, trace=False, ncores=B, **kw):
    st, in_maps = prep(**inputs)
    nc = bacc.Bacc()
    build(nc, st)
    nc.finalize()
    res = run_bass_kernel_spmd(nc, in_maps[:ncores], list(range(ncores)), trace=trace, **kw)
    ctx_np = np.asarray(inputs["context"], np.float32)
    out = np.zeros((B, N, NODE + DEP), np.float32)
    for b in range(ncores):
        chT = np.asarray(res.results[b]["childT"]).astype(np.float32)
        out[b, :, :NODE] = ctx_np[b]
        out[b, :, NODE:] = chT.T
    return out, res


def kernel(**inputs):
    out, _ = run(inputs)
    return out



# revision 4
# speedup vs baseline: 2.5683x; 2.5683x over previous
"""Trainium2 Bass kernel for DepST_RNN (dependency-tree GNN message passing).

Contract: kernel(**inputs) takes FULL inputs, returns FULL output
[B, N, NODE+DEP] float32.  One NeuronCore per sentence (B=8 data-parallel).

V2: matmul-only dataflow — zero SWDGE ops on the critical path.
All indirection (edge gather, scatter-mean, provenance) is baked on host
into one-hot / scaled selection matrices, so every per-layer step is a PE
matmul:
  * uniform slot layout: WR slots per relation per layer (SW = R*WR),
    per-core slot assignment is data (ctxg / oh / Sp / ohf), the
    instruction stream is identical across cores (SPMD, no envelopes).
  * ctx pass: relation-major Wc matmuls over ctxg [256, L*SW] -> mc.
  * per layer l: gather child ct = sum_p chist_p.T @ oh[p,l] (l matmuls),
    40 relation matmuls Wd[r] @ ct[:, r-slots], DVE evac fused with mc add,
    PE transposes to slot-rows, scatter matmul chout = Sp_l.T @ msgS
    (mask/count scaling baked into Sp), evac to compact bf16 chist_l.
  * final: childT = sum_p chist_p.T @ ohf_p, overlapping layer 7.
"""

import sys

sys.path.insert(0, "/opt/trn_rl_repo")

from contextlib import ExitStack

import numpy as np
import ml_dtypes

import concourse.bass as bass
import concourse.bacc as bacc
import concourse.mybir as mybir
from concourse import tile
from concourse.bass_utils import run_bass_kernel_spmd

B, L, E, N = 8, 8, 128, 1024
NODE, DEP, R = 256, 128, 40

BF16 = mybir.dt.bfloat16
F32 = mybir.dt.float32

NPBF16 = ml_dtypes.bfloat16


def prep(context, dep_W, heads, tails, rels, mask):
    """Host-side structure + per-core input tensors."""
    ctx_np = np.asarray(context, np.float32)
    W_np = np.asarray(dep_W, np.float32)
    heads = np.asarray(heads)
    tails = np.asarray(tails)
    rels = np.asarray(rels)
    mask_np = np.asarray(mask, np.float32)

    # uniform relation-slot width across cores/layers
    cnt = np.zeros((B, L, R), np.int64)
    for b in range(B):
        for l in range(L):
            cnt[b, l] = np.bincount(rels[b, l], minlength=R)
    WR = int(cnt.max())
    SW = R * WR                 # slots per layer
    NT = (SW + 127) // 128      # transpose/scatter chunks (last may be partial)
    GW = L * SW                 # ctxg columns: col = r*(L*WR) + l*WR + j
    NOH = L * (L - 1) // 2      # oh tiles: (p, l) p < l, seq = l(l-1)/2 + p

    st = dict(WR=WR, SW=SW, NT=NT, GW=GW, NOH=NOH)

    # shared weight layouts (relation-chunk-major so DMA chunks pipeline)
    wc_np = np.zeros((128, 2 * R * 128), np.float32)   # (2r + c) blocks
    wd_np = np.zeros((128, R * 128), np.float32)
    for r in range(R):
        for c in range(2):
            wc_np[:, (2 * r + c) * 128:(2 * r + c + 1) * 128] = (
                W_np[r, :, c * 128:(c + 1) * 128].T
            )
        wd_np[:, r * 128:(r + 1) * 128] = W_np[r, :, NODE:].T
    wc_np = wc_np.astype(NPBF16)
    wd_np = wd_np.astype(NPBF16)
    ident_np = np.eye(128, dtype=np.float32)

    in_maps = []
    for b in range(B):
        # slot assignment + provenance + compact row maps
        slot = np.zeros((L, E), np.int64)
        for l in range(L):
            c = np.zeros(R, np.int64)
            for e in np.argsort(rels[b, l], kind="stable"):
                r = int(rels[b, l, e])
                slot[l, e] = r * WR + c[r]
                c[r] += 1
        prov = np.full(N, -1, np.int64)
        provs, uidx = [], []
        for l in range(L):
            provs.append(prov.copy())
            hs = sorted(set(heads[b, l].tolist()))
            assert len(hs) <= 128
            uidx.append({h: i for i, h in enumerate(hs)})
            prov[heads[b, l]] = l
        provF = prov

        ctxg = np.zeros((2 * 128, GW), np.float32)
        ohall = np.zeros((128, NOH * SW), np.float32)
        spt = np.zeros((128, L * NT * 128), np.float32)
        ohf = np.zeros((128, L * N), np.float32)
        for l in range(L):
            cv = np.zeros(N, np.float32)
            np.add.at(cv, heads[b, l], mask_np[b, l])
            for e in range(E):
                s = int(slot[l, e])
                g = int(rels[b, l, e]) * (L * WR) + l * WR + (s % WR)
                t, h = int(tails[b, l, e]), int(heads[b, l, e])
                ctxg[:, g] = ctx_np[b, t, :]
                p = int(provs[l][t])
                if p >= 0:
                    ohall[uidx[p][t], (l * (l - 1) // 2 + p) * SW + s] = 1.0
                # Sp lhsT chunk: [slot % 128 partition, (l*NT + s//128)*128 + u]
                u = uidx[l][h]
                spt[s % 128, (l * NT + s // 128) * 128 + u] = (
                    mask_np[b, l, e] / max(float(cv[h]), 1.0)
                )
        for n in range(N):
            p = int(provF[n])
            if p >= 0:
                ohf[uidx[p][n], p * N + n] = 1.0

        in_maps.append(
            dict(
                ctxg=ctxg.astype(NPBF16),
                wc=wc_np,
                wd=wd_np,
                ohall=ohall.astype(NPBF16),
                spt=spt.astype(NPBF16),
                ohf=ohf.astype(NPBF16),
                ident=ident_np,
            )
        )
    return st, in_maps


def build(nc, st):
    WR, SW, NT, GW, NOH = st["WR"], st["SW"], st["NT"], st["GW"], st["NOH"]
    LWR = L * WR  # ctxg relation-block width

    d_ctxg = nc.declare_dram_parameter("ctxg", [256, GW], BF16, isOutput=False)
    d_wc = nc.declare_dram_parameter("wc", [128, 2 * R * 128], BF16, isOutput=False)
    d_wd = nc.declare_dram_parameter("wd", [128, R * 128], BF16, isOutput=False)
    d_oh = nc.declare_dram_parameter("ohall", [128, NOH * SW], BF16, isOutput=False)
    d_spt = nc.declare_dram_parameter("spt", [128, L * NT * 128], BF16, isOutput=False)
    d_ohf = nc.declare_dram_parameter("ohf", [128, L * N], BF16, isOutput=False)
    d_ident = nc.declare_dram_parameter("ident", [128, 128], F32, isOutput=False)
    d_out = nc.declare_dram_parameter("childT", [128, 1024], BF16, isOutput=True)

    NG = 4                      # relation groups for DMA/compute pipelining
    RG = R // NG                # relations per group

    with ExitStack() as ctx:
        tc = ctx.enter_context(tile.TileContext(nc))

        pers = ctx.enter_context(tc.tile_pool(name="pers", bufs=1))

        def sb(name, shape, dt):
            return pers.tile(shape, dt, tag=name, name=name)

        ctxg0 = sb("ctxg0", [128, GW], BF16)
        ctxg1 = sb("ctxg1", [128, GW], BF16)
        wc = sb("wc_sb", [128, 2 * R * 128], BF16)
        wd = sb("wd_sb", [128, R * 128], BF16)
        ohsb = sb("oh_sb", [128, NOH * SW], BF16)
        spt = sb("spt_sb", [128, L * NT * 128], BF16)
        ohfsb = sb("ohf_sb", [128, L * N], BF16)
        ident = sb("ident_sb", [128, 128], F32)
        mcsb = sb("mcsb", [128, GW], BF16)
        chist = sb("chist", [128, L * 128], BF16)
        finT = sb("finT", [128, 1024], BF16)

        pool = ctx.enter_context(tc.tile_pool(name="work", bufs=2))
        pp_wide = ctx.enter_context(tc.tile_pool(name="ps_wide", bufs=2, space="PSUM"))
        pp_ct = ctx.enter_context(tc.tile_pool(name="ps_ct", bufs=1, space="PSUM"))
        pp_md = ctx.enter_context(tc.tile_pool(name="ps_md", bufs=1, space="PSUM"))
        pp_t = ctx.enter_context(tc.tile_pool(name="ps_t", bufs=2, space="PSUM"))
        pp_ch = ctx.enter_context(tc.tile_pool(name="ps_ch", bufs=1, space="PSUM"))

        # ---- input DMAs, interleaved for pipelining ----
        # ctxg+wc per relation-group so ctx matmuls start early
        for g in range(NG):
            a = g * RG * LWR
            w = RG * LWR
            nc.sync.dma_start(ctxg0[:, a:a + w], d_ctxg[0:128, a:a + w])
            nc.sync.dma_start(ctxg1[:, a:a + w], d_ctxg[128:256, a:a + w])
            aw = g * RG * 2 * 128
            ww = RG * 2 * 128
            nc.sync.dma_start(wc[:, aw:aw + ww], d_wc[:, aw:aw + ww])
        nc.sync.dma_start(ident[:, :], d_ident[:, :])
        # layer-0 scatter tables, then wd (layer>=1), then per-layer oh+spt
        nc.sync.dma_start(spt[:, 0:NT * 128], d_spt[:, 0:NT * 128])
        for g in range(NG):
            aw = g * RG * 128
            ww = RG * 128
            nc.sync.dma_start(wd[:, aw:aw + ww], d_wd[:, aw:aw + ww])
        for l in range(1, L):
            a = (l * (l - 1) // 2) * SW
            w = l * SW
            nc.sync.dma_start(ohsb[:, a:a + w], d_oh[:, a:a + w])
            a = l * NT * 128
            nc.sync.dma_start(spt[:, a:a + NT * 128], d_spt[:, a:a + NT * 128])
        nc.sync.dma_start(ohfsb[:, :], d_ohf[:, :])

        # ---- ctx pass: relation-major Wc matmuls into rotating PSUM tiles ----
        # psum tile width 512 = 6.4 relation blocks (LWR=80); emit matmuls per
        # (relation, k-chunk) split at tile boundaries.
        NCT = (GW + 511) // 512
        for t in range(NCT):
            t0, t1 = 512 * t, min(512 * (t + 1), GW)
            ps = pp_wide.tile([128, 512], F32, tag="wide", name=f"msgc{t}")
            r_lo, r_hi = t0 // LWR, (t1 - 1) // LWR
            for r in range(r_lo, r_hi + 1):
                a = max(r * LWR, t0)
                bnd = min((r + 1) * LWR, t1)
                if a >= bnd:
                    continue
                for c in (0, 1):
                    src = ctxg0 if c == 0 else ctxg1
                    nc.tensor.matmul(
                        ps[:, a - t0:bnd - t0],
                        wc[:, (2 * r + c) * 128:(2 * r + c + 1) * 128],
                        src[:, a:bnd],
                        start=(c == 0),
                        stop=(c == 1),
                    )
            nc.vector.tensor_copy(mcsb[:, t0:t1], ps[:, 0:t1 - t0])

        # ---- recursion over layers ----
        for l in range(L):
            if l > 0:
                # gather child: ct = sum_p chist_p.T @ oh[p, l]
                ctp = pp_ct.tile([128, SW], F32, tag="ct", name="ct")
                base = (l * (l - 1) // 2) * SW
                for p in range(l):
                    nc.tensor.matmul(
                        ctp[:, :],
                        chist[:, p * 128:(p + 1) * 128],
                        ohsb[:, base + p * SW:base + (p + 1) * SW],
                        start=(p == 0),
                        stop=(p == l - 1),
                    )
                ctsb = pool.tile([128, SW], BF16, tag="ctsb", name="ctsb")
                nc.vector.tensor_copy(ctsb[:, :], ctp[:, :])
                # relation matmuls: md[:, r-slots] = Wd[r] @ ct[:, r-slots]
                md = pp_md.tile([128, SW], F32, tag="md", name="md")
                for r in range(R):
                    nc.tensor.matmul(
                        md[:, r * WR:(r + 1) * WR],
                        wd[:, r * 128:(r + 1) * 128],
                        ctsb[:, r * WR:(r + 1) * WR],
                        start=True,
                        stop=True,
                    )
            # evac + mc add (mc view: cols r*LWR + l*WR + j), 2 halves
            msum = pool.tile([128, SW], F32, tag="msum", name="msum")
            mcv = mcsb[:, :].rearrange("p (r lw) -> p r lw", lw=LWR)[
                :, :, l * WR:(l + 1) * WR
            ]
            msv = msum[:, :].rearrange("p (r w) -> p r w", w=WR)
            RH = R // 2
            for h in range(2):
                rs = slice(h * RH, (h + 1) * RH)
                if l > 0:
                    mdv = md[:, :].rearrange("p (r w) -> p r w", w=WR)
                    nc.vector.tensor_add(
                        msv[:, rs, :], mdv[:, rs, :], mcv[:, rs, :]
                    )
                else:
                    nc.vector.tensor_copy(msv[:, rs, :], mcv[:, rs, :])
            # transpose to slot-rows + scatter matmul
            chp = pp_ch.tile([128, 128], F32, tag="chout", name="chout")
            for t in range(NT):
                c0, c1 = 128 * t, min(128 * (t + 1), SW)
                cw = c1 - c0
                tp = pp_t.tile([128, 128], F32, tag="tp", name="tp")
                nc.tensor.transpose(tp[0:cw, :], msum[:, c0:c1], ident[:, :])
                msgS = pool.tile([128, 128], BF16, tag=f"msgS{t}", name=f"msgS{t}")
                nc.scalar.copy(msgS[0:cw, :], tp[0:cw, :])
                nc.tensor.matmul(
                    chp[:, :],
                    spt[0:cw, (l * NT + t) * 128:(l * NT + t + 1) * 128],
                    msgS[0:cw, :],
                    start=(t == 0),
                    stop=(t == NT - 1),
                )
            nc.scalar.copy(chist[:, l * 128:(l + 1) * 128], chp[:, :])

        # ---- final: childT = sum_p chist_p.T @ ohf_p ----
        fin0 = pp_wide.tile([128, 512], F32, tag="wide", name="fin0")
        fin1 = pp_wide.tile([128, 512], F32, tag="wide", name="fin1")
        for p in range(L):
            for c, ps in enumerate((fin0, fin1)):
                nc.tensor.matmul(
                    ps[:, :],
                    chist[:, p * 128:(p + 1) * 128],
                    ohfsb[:, p * N + c * 512:p * N + (c + 1) * 512],
                    start=(p == 0),
                    stop=(p == L - 1),
                )
        nc.vector.tensor_copy(finT[:, 0:512], fin0[:, :])
        nc.vector.tensor_copy(finT[:, 512:1024], fin1[:, :])
        nc.sync.dma_start(d_out[:, :], finT[:, :])
    return nc


def run(inputs, trace=False, ncores=B, **kw):
    st, in_maps = prep(**inputs)
    nc = bacc.Bacc()
    build(nc, st)
    nc.finalize()
    res = run_bass_kernel_spmd(nc, in_maps[:ncores], list(range(ncores)), trace=trace, **kw)
    ctx_np = np.asarray(inputs["context"], np.float32)
    out = np.zeros((B, N, NODE + DEP), np.float32)
    for b in range(ncores):
        chT = np.asarray(res.results[b]["childT"]).astype(np.float32)
        out[b, :, :NODE] = ctx_np[b]
        out[b, :, NODE:] = chT.T
    return out, res


def kernel(**inputs):
    out, _ = run(inputs)
    return out
